# revision 37
# baseline (speedup 1.0000x reference)
"""Multi-head multi-resolution location-aware attention on 8 trn2 cores.

Sharding: data-parallel over batch B=32 -> 4 rows/core, all params replicated.
Per core (BL=4, H=4, T=1000, D=1024, DK=DV=512, C=100, FILTS=[25,50,75,100]):
  kT[h,b] = Wk[h].T @ encT_b            (bf16 PE; encT via bf16 DMA-transpose)
  += Watt[h].T @ conv[h,b]              (conv = Toeplitz matmul, fused in PSUM)
  z = tanh(psum + qT col)               (ACT, per-partition bias = q)
  e[t] = g_w[h].T z                     (PE, M=1, accumulated over e-tiles)
  w = softmax(scaling*e) over t         (DVE/ACT on [16, 1000])
  sT[d,hb] = sum_t enc[b,t,d] w[hb,t]   (PE; V never materialized)
  c[b,e] = sT.T @ Wv[h] ; out = cT.T @ Wo
Outputs: out [B,1024] f32, w [H,B,1000] f32.
"""

import numpy as np

H, B, T = 4, 32, 1000
D = 1024  # EPROJS == DUNITS
DK = DV = 512
C = 100
FILTS = [25, 50, 75, 100]
NCORES = 8
BL = B // NCORES  # 4 batch rows per core
HB = H * BL  # 16 (h,b) pairs per core
TP = 1024  # padded T
SCALING = 1.0 / float(np.sqrt(DK))
APL = 2 * 100 + T + 24  # padded att_prev row length


def _build_bass():
    import concourse.bass as bass
    import concourse.bacc as bacc
    import concourse.mybir as mybir
    import concourse.tile as tile
    from concourse.masks import make_identity

    f32 = mybir.dt.float32
    bf16 = mybir.dt.bfloat16
    AF = mybir.ActivationFunctionType

    nc = bacc.Bacc("TRN2", target_bir_lowering=False)

    # ---- I/O ----
    enc = nc.dram_tensor("enc", [BL, T, D], f32, kind="ExternalInput")
    att_pad = nc.dram_tensor("att_pad", [HB, APL], bf16, kind="ExternalInput")
    qt_in = nc.dram_tensor("qt_in", [128, 4, H, BL], f32, kind="ExternalInput")
    gwm_in = nc.dram_tensor("gwm_in", [128, 4, HB, HB], bf16,
                            kind="ExternalInput")
    cwt1_in = nc.dram_tensor("cwt1_in", [128, H, C], bf16,
                             kind="ExternalInput")
    cwt2_in = nc.dram_tensor("cwt2_in", [128, H, C], bf16,
                             kind="ExternalInput")
    Wk = nc.dram_tensor("Wk", [H, D, DK], f32, kind="ExternalInput")
    Wv = nc.dram_tensor("Wv", [H, D, DV], f32, kind="ExternalInput")
    Watt = nc.dram_tensor("Watt", [H, C, DK], f32, kind="ExternalInput")
    Wo = nc.dram_tensor("Wo", [H * DV, D], f32, kind="ExternalInput")
    out_t = nc.dram_tensor("out", [BL, D], f32, kind="ExternalOutput")
    w_t = nc.dram_tensor("w", [H, BL, T], f32, kind="ExternalOutput")

    with tile.TileContext(nc) as tc:
        with (
            tc.tile_pool(name="singles", bufs=1) as singles,
            tc.tile_pool(name="io", bufs=1) as iop,
            tc.tile_pool(name="wkp", bufs=1) as wkp,
            tc.tile_pool(name="wvop", bufs=1) as wvop,
            tc.tile_pool(name="toep", bufs=6) as toep,
            tc.tile_pool(name="zp", bufs=6) as zp,
            tc.tile_pool(name="convsb", bufs=3) as convsb,
            tc.tile_pool(name="smx", bufs=2) as smx,
            tc.tile_pool(name="psA", bufs=4, space="PSUM") as psA,
            tc.tile_pool(name="psE", bufs=1, space="PSUM") as psE,
            tc.tile_pool(name="psM", bufs=2, space="PSUM") as psM,
        ):
            # ---------- constants ----------
            ident = singles.tile([128, 128], bf16)
            make_identity(nc, ident)
            ident32 = singles.tile([32, 32], f32)
            make_identity(nc, ident32)

            # ---------- loads, ordered for earliest PE start ----------
            qt = singles.tile([128, 4, H, BL], f32)
            nc.sync.dma_start(out=qt, in_=qt_in[:])
            gwm = singles.tile([128, 4, HB, HB], bf16)
            nc.sync.dma_start(out=gwm, in_=gwm_in[:])
            cwt1 = singles.tile([128, H, C], bf16)
            nc.sync.dma_start(out=cwt1, in_=cwt1_in[:])
            cwt2 = singles.tile([128, H, C], bf16)
            nc.sync.dma_start(out=cwt2, in_=cwt2_in[:])
            ksplit = [(min(2 * f + 1, 128), max(0, 2 * f + 1 - 128))
                      for f in FILTS]
            # enc b0 first, then per-head Wk, then enc b1-3
            etmps = [iop.tile([128, 8, D], bf16, tag=f"io{b}", name=f"etmp{b}")
                     for b in range(BL)]

            def load_etmp(b):
                if b == 0:
                    nc.gpsimd.dma_start(
                        out=etmps[b][:, 0:2, :],
                        in_=enc[b, 0:256, :].rearrange("(tt p) d -> p tt d",
                                                       p=128))
                    nc.gpsimd.dma_start(
                        out=etmps[b][:, 2:4, :],
                        in_=enc[b, 256:512, :].rearrange("(tt p) d -> p tt d",
                                                         p=128))
                else:
                    nc.gpsimd.dma_start(
                        out=etmps[b][:, 0:4, :],
                        in_=enc[b, 0:512, :].rearrange("(tt p) d -> p tt d",
                                                       p=128))
                nc.gpsimd.dma_start(
                    out=etmps[b][:, 4:7, :],
                    in_=enc[b, 512:896, :].rearrange("(tt p) d -> p tt d", p=128))
                nc.gpsimd.dma_start(out=etmps[b][:104, 7, :],
                                    in_=enc[b, 896:T, :])

            load_etmp(0)
            wk_sb = wkp.tile([128, H, 8, DK], bf16, tag="wbig")
            for h in range(H):
                nc.gpsimd.dma_start(
                    out=wk_sb[:, h, :, :],
                    in_=Wk[h].rearrange("(dt p) e -> p dt e", p=128))
            watt_sb = singles.tile([128, H, DK], bf16)  # [c(100), h, e]
            for h in range(H):
                nc.gpsimd.dma_start(out=watt_sb[:C, h, :], in_=Watt[h])
            for b in range(1, BL):
                load_etmp(b)
            wo_sb = wvop.tile([128, 16, D], bf16, tag="wbig2")
            nc.gpsimd.dma_start(
                out=wo_sb, in_=Wo[:].rearrange("(ht p) d -> p ht d", p=128))

            # ---------- K-proj + conv + energies main loop ----------
            ap_ap = att_pad[:]

            def toep_ap(hb, h, p0, nrow):
                # row p of the tile reads att_pad[hb, (100-f)+p0+p : ... +TP]
                start = (100 - FILTS[h]) + p0
                return bass.AP(
                    tensor=ap_ap.tensor,
                    offset=ap_ap.offset + hb * APL + start,
                    ap=[[1, nrow], [1, TP]])

            wt = singles.tile([128, 8, HB], bf16)
            st = singles.tile([128, 8, HB], bf16)
            w_bufs = {}

            def softmax_b(b, peE):
                T1 = T - 512  # valid cols in chunk 1
                mx0 = smx.tile([HB, 1], f32, tag="mx0", name=f"mx0{b}")
                nc.vector.reduce_max(out=mx0, in_=peE[0],
                                     axis=mybir.AxisListType.X)
                mx1 = smx.tile([HB, 1], f32, tag="mx1", name=f"mx1{b}")
                nc.vector.reduce_max(out=mx1, in_=peE[1][:, 0:T1],
                                     axis=mybir.AxisListType.X)
                nmxb = smx.tile([HB, 1], f32, tag="nmxb", name=f"nmxb{b}")
                nc.vector.tensor_max(out=nmxb, in0=mx0, in1=mx1)
                nc.scalar.mul(nmxb, nmxb, -SCALING)
                wEb = smx.tile([HB, TP], f32, tag="wEb", name=f"wEb{b}")
                nc.scalar.activation(out=wEb[:, 0:512], in_=peE[0],
                                     func=AF.Exp, bias=nmxb, scale=SCALING)
                nc.scalar.activation(out=wEb[:, 512:T], in_=peE[1][:, 0:T1],
                                     func=AF.Exp, bias=nmxb, scale=SCALING)
                smb = smx.tile([HB, 1], f32, tag="smb", name=f"smb{b}")
                nc.vector.reduce_sum(out=smb, in_=wEb[:, 0:T],
                                     axis=mybir.AxisListType.X)
                rsb = smx.tile([HB, 1], f32, tag="rsb", name=f"rsb{b}")
                nc.vector.reciprocal(out=rsb, in_=smb)
                wf = smx.tile([HB, TP], f32, tag="wf", name=f"wf{b}")
                nc.vector.memset(wf[:, T:TP], 0.0)
                nc.vector.tensor_scalar_mul(wf[:, 0:T], wEb[:, 0:T], rsb)
                nc.sync.dma_start(
                    out=w_t[:, b, :].rearrange("h t -> h t"),
                    in_=wf[b::BL, 0:T])
                w_bufs[b] = wf

            def pe_tail(b, pcs=None):
                wf = w_bufs.pop(b)
                for tt in range(8):
                    pt = psM.tile([128, 128], f32, tag="m",
                                  name=f"wtp{b}_{tt}")
                    nc.tensor.transpose(pt[:, :HB],
                                        wf[:, tt * 128:(tt + 1) * 128],
                                        ident32[:HB, :HB])
                    nc.vector.tensor_copy(out=wt[:, tt, b::BL],
                                          in_=pt[:, b:b + 1 + 3 * BL:BL])
                for dm in range(8):
                    ps = psM.tile([128, 512], f32, tag="m")
                    for tt in range(8):
                        rows = 128 if tt < 7 else 104
                        nc.tensor.matmul(
                            ps[:, :BL],
                            etmps[b][:rows, tt, dm * 128:(dm + 1) * 128],
                            wt[:rows, tt, b::BL],
                            start=(tt == 0), stop=(tt == 7))
                    nc.vector.tensor_copy(out=st[:, dm, b::BL],
                                          in_=ps[:, :BL])
                    if pcs is not None:
                        for h in range(H):
                            nc.tensor.matmul(pcs[h][:BL, :],
                                             st[:, dm, h * BL:(h + 1) * BL],
                                             wv_sb[:, h, dm, :],
                                             start=(dm == 0), stop=(dm == 7))

            for b in range(BL):
                peE = [psE.tile([HB, 512], f32, tag=f"E{ch}",
                                name=f"peE{b}_{ch}") for ch in range(2)]
                etmp = etmps[b]
                enct = [iop.tile([128, TP], bf16, tag=f"tt{dt}",
                                 name=f"enct{b}_{dt}") for dt in range(8)]
                for half in range(2):
                    ttdt = ([(tt, dt) for tt in range(half * 4, half * 4 + 4)
                             for dt in range(8)] if b == 0 else
                            [(tt, dt) for dt in range(8)
                             for tt in range(half * 4, half * 4 + 4)])
                    for tt, dt in ttdt:
                            ptr = psM.tile([128, 128], bf16, tag="m",
                                           name=f"ptr{b}_{dt}_{tt}")
                            nc.tensor.transpose(ptr, etmp[:, tt,
                                                dt * 128:(dt + 1) * 128],
                                                ident)
                            nc.vector.tensor_copy(
                                out=enct[dt][:, tt * 128:(tt + 1) * 128],
                                in_=ptr)
                for h in range(H):
                    hb = h * BL + b
                    k1, k2 = ksplit[h]
                    tp1 = toep.tile([128, TP], bf16, tag="toep")
                    nc.sync.dma_start(out=tp1[:k1, :],
                                      in_=toep_ap(hb, h, 0, k1))
                    tp2 = None
                    if k2 > 0:
                        tp2 = toep.tile([128, TP], bf16, tag="toep")
                        nc.sync.dma_start(out=tp2[:k2, :],
                                          in_=toep_ap(hb, h, 128, k2))
                    for ch in range(2):
                        tsl = slice(ch * 512, (ch + 1) * 512)
                        pconv = psA.tile([128, 512], f32, tag="k")
                        nc.tensor.matmul(pconv[:C, :], cwt1[:k1, h, :],
                                         tp1[:k1, tsl], start=True,
                                         stop=(k2 == 0))
                        if k2 > 0:
                            nc.tensor.matmul(pconv[:C, :], cwt2[:k2, h, :],
                                             tp2[:k2, tsl], start=False,
                                             stop=True)
                        csb = convsb.tile([128, 512], bf16, tag="csb")
                        nc.vector.tensor_copy(out=csb[:C, :], in_=pconv[:C, :])
                        for et in range(4):
                            esl = slice(et * 128, (et + 1) * 128)
                            pk = psA.tile([128, 512], f32, tag="k")
                            for dt in range(8):
                                nc.tensor.matmul(pk, wk_sb[:, h, dt, esl],
                                                 enct[dt][:, tsl],
                                                 start=(dt == 0), stop=False)
                            nc.tensor.matmul(pk, watt_sb[:C, h, esl],
                                             csb[:C, :], start=False, stop=True)
                            z = zp.tile([128, 512], bf16, tag="z")
                            nc.scalar.activation(out=z, in_=pk, func=AF.Tanh,
                                                 bias=qt[:, et, h, b:b + 1],
                                                 scale=1.0)
                            nc.tensor.matmul(
                                peE[ch], gwm[:, et, hb, :], z,
                                start=(h == 0 and et == 0),
                                stop=(h == H - 1 and et == 3))

                softmax_b(b, peE)
                if b > 0:
                    pe_tail(b - 1)
                if b == 1:
                    # Wv reuses Wk slots per-head; loads fire as heads retire
                    wv_sb = wkp.tile([128, H, 8, DV], bf16, tag="wbig")
                    for h in range(H):
                        nc.gpsimd.dma_start(
                            out=wv_sb[:, h, :, :],
                            in_=Wv[h].rearrange("(dt p) e -> p dt e", p=128))
            pcs = [psA.tile([128, 512], f32, tag="k", name=f"pc{h}")
                   for h in range(H)]
            pe_tail(BL - 1, pcs=pcs)

            # ---------- cT + final, interleaved ----------
            c_sb = singles.tile([BL, H, DV], bf16)
            ct = singles.tile([128, 16, BL], bf16)    # [e_p, (h,et), b]
            po = [psE.tile([HB, 512], f32, tag=f"E{half}", name=f"po{half}")
                  for half in range(2)]
            for h in range(H):
                nc.vector.tensor_copy(out=c_sb[:, h, :], in_=pcs[h][:BL, :])
                for et in range(4):
                    ht = h * 4 + et
                    pt = psM.tile([128, 128], bf16, tag="m")
                    nc.tensor.transpose(pt[:, :BL],
                                        c_sb[:, h, et * 128:(et + 1) * 128],
                                        ident[:BL, :BL])
                    nc.vector.tensor_copy(out=ct[:, ht, :], in_=pt[:, :BL])
                    for half in range(2):
                        nc.tensor.matmul(
                            po[half][:BL, :], ct[:, ht, :],
                            wo_sb[:, ht, half * 512:(half + 1) * 512],
                            start=(ht == 0), stop=(ht == 15))
            osb = singles.tile([BL, D], f32)
            for half in range(2):
                nc.vector.tensor_copy(out=osb[:, half * 512:(half + 1) * 512],
                                      in_=po[half][:BL, :])
            nc.sync.dma_start(out=out_t[:], in_=osb)

    nc.compile()
    return nc


_NC_CACHE = None


def kernel(**inputs):
    global _NC_CACHE
    from concourse import bass_utils

    if _NC_CACHE is None:
        _NC_CACHE = _build_bass()
    nc = _NC_CACHE

    in_maps = []
    for c in range(NCORES):
        bs = slice(c * BL, (c + 1) * BL)
        m = {
            "enc": np.ascontiguousarray(inputs["enc_hs_pad"][bs]),
            "Wk": inputs["Wk"], "Wv": inputs["Wv"], "Watt": inputs["Watt"],
            "Wo": inputs["Wo"],
        }
        m = {k: np.ascontiguousarray(np.asarray(v, dtype=np.float32))
             for k, v in m.items()}
        import ml_dtypes
        bf = ml_dtypes.bfloat16
        apd = np.zeros((HB, APL), dtype=bf)
        apd[:, 100:100 + T] = np.asarray(
            inputs["att_prev"][:, bs], np.float32).reshape(HB, T).astype(bf)
        m["att_pad"] = apd
        # q projection on host (f32): q[h,b,e] = dec_z[b] @ Wq[h] + bq[h]
        dz = np.asarray(inputs["dec_z"][bs], np.float32)
        Wq = np.asarray(inputs["Wq"], np.float32)
        bqv = np.asarray(inputs["bq"], np.float32)
        q = np.einsum("bd,hde->hbe", dz, Wq) + bqv[:, None, :]  # [H,BL,DK]
        # qt[p, et, h, b] = q[h, b, et*128+p]
        m["qt_in"] = np.ascontiguousarray(
            q.reshape(H, BL, 4, 128).transpose(3, 2, 0, 1).astype(np.float32))
        # column-masked g_w: gwm[p, et, hb, col] = g_w[hb//BL, et*128+p]*(col==hb)
        gw = np.asarray(inputs["g_w"], np.float32).reshape(H, 4, 128)
        gwm = np.zeros((128, 4, HB, HB), np.float32)
        for hb in range(HB):
            gwm[:, :, hb, hb] = gw[hb // BL].T
        m["gwm_in"] = gwm.astype(bf)
        # transposed conv filters, split at tap 128
        cwt1 = np.zeros((128, H, C), np.float32)
        cwt2 = np.zeros((128, H, C), np.float32)
        for h in range(H):
            kh = 2 * FILTS[h] + 1
            cw = np.asarray(inputs[f"conv_w{h}"], np.float32).reshape(C, kh)
            k1 = min(kh, 128)
            cwt1[:k1, h, :] = cw[:, :k1].T
            if kh > 128:
                cwt2[:kh - 128, h, :] = cw[:, 128:].T
        m["cwt1_in"] = cwt1.astype(bf)
        m["cwt2_in"] = cwt2.astype(bf)
        in_maps.append(m)

    global _last_in_maps
    _last_in_maps = in_maps
    res = bass_utils.run_bass_kernel_spmd(nc, in_maps,
                                          core_ids=list(range(NCORES)))
    out = np.concatenate([r["out"] for r in res.results], axis=0)
    w = np.concatenate([r["w"] for r in res.results], axis=1)
    return out.astype(np.float32), w.astype(np.float32)
